# revision 64
# baseline (speedup 1.0000x reference)
"""DCGRU cell Trainium2 kernel (fp8 DoubleRow aggregation).

Math (per batch i):
  xs = [input, state]                                  [N, 66]
  aggr[j] = S[j] @ xs          (J=4 supports)          [N, 66]
  r = sigmoid(sum_j aggr[j] @ Wr[j] + br)              [N, 64]
  um = 1 - sigmoid(sum_j aggr[j] @ Wu[j] + bu)         (= 1-u, via scale=-1)
  xc = [input, r*state]
  c = tanh(sum_j (S[j] @ xc) @ Wc[j] + bc)
  out = (1-um)*state + um*c = state + um*(c - state)

Sharding: data-parallel over batch, 8 batches per core on 8 cores.
supports/weights replicated. No collectives.

Device kernel (per core, Bc=8):
  - 3-product compensated fp8 aggregation (full-fp8 alone misses the
    2e-2 gate at ~8e-2): S ~ (Sh + Sl/16)/64, x ~ xh + xl/16, with
    psum = Sh16@xh (DoubleRow m-pairs, slot0) + (Sh@xl + Sl@xh) (one
    DoubleRow instruction per m-tile, slots), all e4m3. 24 instructions
    per [128, 264] psum half vs 16 for fp16 -- 25% less PE time at
    near-fp16 accuracy (max rel ~2.5e-3). Drains scale by 1/16; the
    remaining 1/64 is folded into the fp16 weights on host.
  - Supports host-swizzled [j, g, mh, p, mt, slot(Sh16|Sh|Sl), k] so
    every stationary half-tile DMA is fully contiguous per partition.
  - xs packed [p, h, mt, slot(xl|xh), 264] fp8, two DMAs.
  - Tail per group g (pipelined one group behind agg): per batch i issue
    act(i-2) -> transposes(i) + drains -> proj(i-1), so PE never waits on
    the cross-engine drain/act latency.
  - post_act hook spreads phase-boundary and combine work across groups:
    phase 1: (r*state).T chunk -> fp8 hi/lo split (DVE, same-queue
    chain) -> transposes -> xc fp8 slots;
    phase 2: out chunk = state + um*(c - state), DMA at last group.
  - Engine queue discipline throughout: all PSUM drains on DVE/Scalar
    (GPSIMD cannot access PSUM on hw); cross-engine dependency chains
    stay within one queue so an in-order queue head never blocks a
    latency-critical drain; issue order ~= execution order per queue
    (agg j-blocks interleaved with two tail iterations each).
"""

import sys

if '/opt/trn_rl_repo' not in sys.path:
    sys.path.insert(0, '/opt/trn_rl_repo')

import numpy as np
import ml_dtypes

B, N, IN, OUT, J = 64, 2048, 2, 64, 4
NCORES = 8
BC = B // NCORES            # 8 batches per core
F = IN + OUT                # 66
CB = BC * F                 # 528 moving columns
P = 128
HALF = CB // 2              # 264 (psum bank split)
NG = 8                      # k groups (256 cols each)
KBG = 2                     # k blocks per group
GW = KBG * P                # 256 cols per group
MQ = 8                      # m pair-tiles (each pair = 256 rows)
SCALE = 64.0                # fp8 support prescale (folded out via W/SCALE)

_CACHE = {}


def _build_module():
    import concourse.tile as tile
    import concourse.mybir as mybir
    from concourse import bacc
    from concourse.masks import make_identity

    f32 = mybir.dt.float32
    f16 = mybir.dt.float16
    f8 = mybir.dt.float8e4
    AF = mybir.ActivationFunctionType
    PM = mybir.MatmulPerfMode.DoubleRow

    nc = bacc.Bacc("TRN2", target_bir_lowering=False, debug=False,
                   num_devices=1)

    st_d = nc.dram_tensor("st", [J, NG, 2, P, MQ, 3, GW], f8,
                          kind="ExternalInput").ap()
    xs_d = nc.dram_tensor("xs", [P, 2, 2 * MQ, 2, HALF], f8,
                          kind="ExternalInput").ap()
    stT_d = nc.dram_tensor("stT", [BC, OUT, N], f16, kind="ExternalInput").ap()
    wcat_d = nc.dram_tensor("wcat", [F, J * 2 * OUT + J * OUT], f16,
                            kind="ExternalInput").ap()
    bvec_d = nc.dram_tensor("bvec", [P, 3], f32, kind="ExternalInput").ap()
    outT_d = nc.dram_tensor("outT", [BC, OUT, N], f16,
                            kind="ExternalOutput").ap()

    with tile.TileContext(nc) as tc:
        with tc.tile_pool(name="const", bufs=1) as const_pool, \
             tc.tile_pool(name="xs", bufs=1) as xs_pool, \
             tc.tile_pool(name="xc", bufs=1) as xc_pool, \
             tc.tile_pool(name="ruT", bufs=BC) as ruT_pool, \
             tc.tile_pool(name="stT", bufs=BC) as stT_pool, \
             tc.tile_pool(name="um0", bufs=BC) as um0_pool, \
             tc.tile_pool(name="rsT", bufs=2) as rsT_pool, \
             tc.tile_pool(name="stst", bufs=8) as st_pool, \
             tc.tile_pool(name="agg", bufs=16) as agg_pool, \
             tc.tile_pool(name="aggT", bufs=4) as aggT_pool, \
             tc.tile_pool(name="aggps", bufs=2, space="PSUM") as agg_ps_pool, \
             tc.tile_pool(name="tpps", bufs=4, space="PSUM") as tp_ps_pool, \
             tc.tile_pool(name="pjps", bufs=2, space="PSUM") as pj_ps_pool, \
             tc.tile_pool(name="cscr", bufs=2) as cscr_pool:

            # xs issued inside phase-1's pre_fn (after the first support tile)
            # in 4 chunks so the first m-pair matmuls start ~7us earlier.
            xs_t = xs_pool.tile([P, 2, 2 * MQ, 2, HALF], f8, tag="xs")

            def xs_load():
                # between st_a(0,0) and st_b(0,0): first xs chunk
                nc.sync.dma_start(xs_t[:, 0, 0:MQ], xs_d[:, 0, 0:MQ])

            def xs_load_rest():
                # after st_b(0,0): remaining xs chunks
                nc.sync.dma_start(xs_t[:, 0, MQ:], xs_d[:, 0, MQ:])
                nc.sync.dma_start(xs_t[:, 1, 0:MQ], xs_d[:, 1, 0:MQ])
                nc.sync.dma_start(xs_t[:, 1, MQ:], xs_d[:, 1, MQ:])
            wcat_t = const_pool.tile([F, J * 2 * OUT + J * OUT], f16,
                                     tag="wcat")
            bvec_t = const_pool.tile([P, 3], f32, tag="bvec")

            ident = const_pool.tile([P, P], f16, tag="ident")
            make_identity(nc, ident[:])

            wru_t = [wcat_t[:, j * 2 * OUT:(j + 1) * 2 * OUT]
                     for j in range(J)]
            wc_t = [wcat_t[:, J * 2 * OUT + j * OUT:J * 2 * OUT + (j + 1) * OUT]
                    for j in range(J)]
            bru_ap = bvec_t[:, 0:1]
            sgn_ap = bvec_t[:, 1:2]
            bc_ap = bvec_t[0:OUT, 2:3]

            xc_t = xc_pool.tile([P, 2, 2 * MQ, 2, HALF], f8, tag="xc")

            def xc_input_copy():
                # emitted in phase1_extra(0), after the xs DMAs exist
                for h in range(2):
                    dst = xc_t[:, h].rearrange("p m two (i f) -> p m two i f",
                                               f=F)
                    src = xs_t[:, h].rearrange("p m two (i f) -> p m two i f",
                                               f=F)
                    nc.gpsimd.tensor_copy(dst[:, :, :, :, 0:IN],
                                          src[:, :, :, :, 0:IN])

            stT_tiles = [stT_pool.tile([OUT, N], f16, tag="stT",
                                       name=f"stT{i}")
                         for i in range(BC)]
            ruT_tiles = [ruT_pool.tile([P, N], f16, tag="ruT", name=f"ruT{i}")
                         for i in range(BC)]
            um0_tiles = [um0_pool.tile([OUT, N], f16, tag="um0",
                                       name=f"um0{i}")
                         for i in range(BC)]

            def big_phase(x_t, w_tiles, out_rows, bias_ap, act_fn,
                          scale_ap, out_slice_fn, agg_rr, aggT_rr,
                          extra_fn=None, post_act=None, tp_bufs=4,
                          pre_fn=None, post_fn=None, defer_first=False,
                          split_h=False, split_last=False):
                """One graph-conv pass + projection + activation.

                Pipelined: agg(g) is issued before tail(g-1); within a tail,
                act(i-2) -> post_act(i-2) -> transposes(i)+drains ->
                proj(i-1). Queue discipline: gp takes the slack-tolerant agg
                psum drains; scalar/DVE take the latency-critical tail work;
                post_act chains stay within one queue so they never block a
                drain at an in-order queue head.
                """
                with tc.tile_pool(name="agg", bufs=16) as agg_pool, \
                     tc.tile_pool(name="aggT", bufs=4) as aggT_pool, \
                     tc.tile_pool(name="aggps", bufs=3, space="PSUM") as agg_ps_pool, \
                     tc.tile_pool(name="tpps", bufs=tp_bufs, space="PSUM") as tp_ps_pool, \
                     tc.tile_pool(name="pjps", bufs=pj_bufs, space="PSUM") as pj_ps_pool:

                    nd = 0
                    td = 0
                    pend_proj = [None]
                    pend_act = [None]

                    def agg_block(g, j, pre_fn=None, post_fn=None,
                                  defer_m=False, hs=(0, 1)):
                        # 3-product compensated fp8 aggregation for one
                        # support: psum = Sh16@xh (m-pairs, slot0) +
                        # (Sh@xl + Sl@xh) per m-tile (slots 1,2 x 0,1);
                        # drain scales by 1/16 -> aggr = 64*S@x in f16.
                        nonlocal nd
                        blk = {}
                        st_a = st_pool.tile([P, MQ, 3, GW], f8, tag="st")
                        nc.sync.dma_start(st_a[:], st_d[j, g, 0])
                        if pre_fn is not None:
                            pre_fn()
                        st_b = st_pool.tile([P, MQ, 3, GW], f8, tag="st")
                        nc.sync.dma_start(st_b[:], st_d[j, g, 1])
                        if post_fn is not None:
                            post_fn()
                        st_h = [st_a, st_b]
                        tl = {}
                        for kb in range(KBG):
                            tl[kb] = agg_pool.tile([P, CB], f16, tag="agg",
                                                   name=f"agg{j}_{kb}")
                            blk[(j, kb)] = tl[kb]
                        emit_half(j, st_h, tl, defer_m, hs=hs)
                        return blk, st_h, tl

                    def emit_half(j, st_h, tl, defer_m, hs=(0, 1)):
                        # h outer: both kb's h=0 psums before any h=1 work,
                        # so the late xs chunks (h=1) are off the critical
                        # path at startup. defer_m pushes the last m-pair to
                        # the end of each accumulation (its moving data is
                        # written by the final phase-1 post_act when this is
                        # phase 2's first block).
                        nonlocal nd

                        def mm(pst, h, kind, m, kb, start, stop):
                            if kind == 'q':
                                stat = st_h[m // (MQ // 2)][
                                    :, 2 * (m % (MQ // 2)):
                                    2 * (m % (MQ // 2)) + 2, 0,
                                    kb * P:(kb + 1) * P]
                                mov = x_t[:, h, 2 * m:2 * m + 2, 1, :]
                            else:
                                stat = st_h[m // MQ][:, m % MQ, 1:3,
                                                     kb * P:(kb + 1) * P]
                                mov = x_t[:, h, m, :, :]
                            nc.tensor.matmul(pst[:], stat, mov, start=start,
                                             stop=stop, perf_mode=PM)

                        early = ([('q', mq) for mq in range(MQ - 1)]
                                 + [('t', mt) for mt in range(2 * MQ - 2)])
                        late = [('q', MQ - 1), ('t', 2 * MQ - 2),
                                ('t', 2 * MQ - 1)]
                        full = ([('q', mq) for mq in range(MQ)]
                                + [('t', mt) for mt in range(2 * MQ)])
                        for h in hs:
                            for kb in range(KBG):
                                if defer_m:
                                    pst = agg_ps_pool.tile(
                                        [P, HALF], f32, tag="aggps",
                                        name=f"aggps{kb}_{h}")
                                    for n, (kind, m) in enumerate(early):
                                        mm(pst, h, kind, m, kb, n == 0,
                                           False)
                                    for n, (kind, m) in enumerate(late):
                                        mm(pst, h, kind, m, kb, False,
                                           n == len(late) - 1)
                                    eng = agg_rr(nd)
                                    nd += 1
                                    eng(tl[kb][:, h * HALF:(h + 1) * HALF],
                                        pst[:])
                                    continue
                                pst = agg_ps_pool.tile(
                                    [P, HALF], f32, tag="aggps",
                                    name=f"aggps{kb}_{h}")
                                for n, (kind, m) in enumerate(full):
                                    mm(pst, h, kind, m, kb, n == 0,
                                       n == len(full) - 1)
                                eng = agg_rr(nd)
                                nd += 1
                                eng(tl[kb][:, h * HALF:(h + 1) * HALF],
                                    pst[:])

                    def one_act(g, split=False):
                        pi, ppp = pend_act[0]
                        if split:
                            # final act: per-half act+post chain shortens the
                            # end-of-kernel critical path
                            pend_act[0] = None
                            for kb in range(KBG):
                                nc.scalar.activation(
                                    out_slice_fn(pi, g * GW + kb * P, P),
                                    ppp[:, kb * P:(kb + 1) * P],
                                    act_fn, bias=bias_ap, scale=scale_ap)
                                if post_act is not None:
                                    post_act(pi, g, tp_ps_pool, kb=kb)
                            return
                        nc.scalar.activation(
                            out_slice_fn(pi, g * GW, GW), ppp[:],
                            act_fn, bias=bias_ap, scale=scale_ap)
                        pend_act[0] = None
                        if post_act is not None:
                            post_act(pi, g, tp_ps_pool)

                    def issue_proj(g):
                        pi, tpair = pend_proj[0]
                        pp = pj_ps_pool.tile([out_rows, GW], f32, tag="pj",
                                             name=f"proj{pi}")
                        for j in range(J):
                            nc.tensor.matmul(
                                pp[:], w_tiles[j][:, 0:out_rows],
                                tpair[j // 2][:, j % 2],
                                start=(j == 0), stop=(j == J - 1))
                        pend_proj[0] = None
                        pend_act[0] = (pi, pp)

                    def tail_iter(g, i, agg_sb, flush=False):
                        # during the final flush the agg psum banks are idle:
                        # borrow them for extra transpose capacity; also put
                        # the PE transposes ahead of the act so its post_act
                        # chain doesn't delay them
                        nonlocal td
                        if pend_act[0] is not None and not flush:
                            one_act(g)      # act(i-2)
                        tpair = []
                        for half in range(2):
                            pool = agg_ps_pool if (flush and half == 0) \
                                else tp_ps_pool
                            tp = pool.tile([F, 2, GW], f16,
                                           tag="tppair" if pool is tp_ps_pool
                                           else "aggps",
                                           name=f"tp{i}_{half}")
                            for jj in range(2):
                                j = half * 2 + jj
                                for kb in range(KBG):
                                    nc.tensor.transpose(
                                        tp[:, jj, kb * P:(kb + 1) * P],
                                        agg_sb[(j, kb)][:, i * F:(i + 1) * F],
                                        ident[:])
                            at = aggT_pool.tile([F, 2, GW], f16, tag="aggT",
                                                name=f"aggT{i}_{half}")
                            aggT_rr[td % len(aggT_rr)](at[:], tp[:])
                            td += 1
                            tpair.append(at)
                        if pend_act[0] is not None and flush:
                            one_act(g)
                        if pend_proj[0] is not None:
                            issue_proj(g)   # proj(i-1)
                        pend_proj[0] = (i, tpair)

                    def tail_flush(g, last=False):
                        # act(BC-2), proj(BC-1), act(BC-1)
                        if pend_act[0] is not None:
                            one_act(g)
                        issue_proj(g)
                        one_act(g, split=last and split_last)

                    # Interleaved issue: each engine queue sees work in
                    # (approximate) temporal execution order, so no queue
                    # head ever waits on far-future work while near-term
                    # work sits behind it.
                    agg_prev = None
                    for g in range(NG):
                        agg_cur = {}
                        if g == 0 and split_h:
                            # all four supports' h=0 psums first: they need
                            # only the first two xs chunks, so the h=1 xs
                            # DMAs are off the startup critical path.
                            held = []
                            for j in range(J):
                                first = j == 0
                                blk, st_h, tl = agg_block(
                                    0, j,
                                    pre_fn if first else None,
                                    post_fn if first else None,
                                    hs=(0,))
                                agg_cur.update(blk)
                                held.append((j, st_h, tl))
                                if extra_fn is not None:
                                    extra_fn(0, j)
                            for j, st_h, tl in held:
                                emit_half(j, st_h, tl, False, hs=(1,))
                            agg_prev = agg_cur
                            continue
                        for j in range(J):
                            first = g == 0 and j == 0
                            blk, _, _ = agg_block(
                                g, j,
                                pre_fn if first else None,
                                post_fn if first else None,
                                defer_m=(g == 0 and defer_first))
                            agg_cur.update(blk)
                            if extra_fn is not None:
                                extra_fn(g, j)
                            if agg_prev is not None:
                                tail_iter(g - 1, 2 * j, agg_prev)
                                tail_iter(g - 1, 2 * j + 1, agg_prev)
                        if agg_prev is not None:
                            tail_flush(g - 1)
                        agg_prev = agg_cur
                    for i in range(BC):
                        tail_iter(NG - 1, i, agg_prev, flush=True)
                    tail_flush(NG - 1, last=True)

            # ---------------- phase 1 ----------------
            def phase1_extra(g, j):
                if g == 0 and j == 0:
                    xc_input_copy()
                    nc.sync.dma_start(wcat_t[:], wcat_d[:])
                    nc.sync.dma_start(bvec_t[:], bvec_d[:])
                # two stT tiles per (g, j) slot from (0,2) on: all 8 landed
                # by the start of g1's tails (first consumer ~post_act(i,0)).
                slot = g * J + j
                if 3 <= slot <= 6:
                    for i in range((slot - 3) * 2, (slot - 3) * 2 + 2):
                        nc.sync.dma_start(stT_tiles[i][:, 0:N // 2],
                                          stT_d[i][:, 0:N // 2])
                elif 7 <= slot <= 10:
                    for i in range((slot - 7) * 2, (slot - 7) * 2 + 2):
                        nc.sync.dma_start(stT_tiles[i][:, N // 2:],
                                          stT_d[i][:, N // 2:])

            rst_n = [0]

            def phase1_post_act(i, g, tp_pool, kb=None):
                # (r*state).T chunk, split hi/lo, transposed into xc fp8:
                #   rsh8 = fp8(rst); res = rst - f16(rsh8); xl = fp8(16*res)
                # hi and res transposed separately; drains convert to fp8.
                k0 = g * GW
                h, il = divmod(i, 4)
                rst = rsT_pool.tile([OUT, GW], f16, tag="rsT")
                nc.vector.tensor_mul(rst[:], ruT_tiles[i][0:OUT, k0:k0 + GW],
                                     stT_tiles[i][:, k0:k0 + GW])
                rsh8 = rsT_pool.tile([OUT, GW], f8, tag="rs8")
                nc.vector.tensor_copy(rsh8[:], rst[:])
                rsh16 = rsT_pool.tile([OUT, GW], f16, tag="rsH")
                nc.vector.tensor_copy(rsh16[:], rsh8[:])
                res = rsT_pool.tile([OUT, GW], f16, tag="rsR")
                nc.vector.tensor_sub(res[:], rst[:], rsh16[:])
                tph = tp_pool.tile([P, KBG, OUT], f16, tag="tppair",
                                   name=f"rsth{i}")
                tpl = tp_pool.tile([P, KBG, OUT], f16, tag="tppair",
                                   name=f"rstl{i}")
                for kb in range(KBG):
                    nc.tensor.transpose(tph[:, kb],
                                        rsh16[:, kb * P:(kb + 1) * P],
                                        ident[0:OUT, 0:OUT])
                    nc.tensor.transpose(tpl[:, kb],
                                        res[:, kb * P:(kb + 1) * P],
                                        ident[0:OUT, 0:OUT])
                for kb in range(KBG):
                    mt = g * KBG + kb
                    ds = xc_t[:, h, mt, :, il * F + IN:(il + 1) * F]
                    if rst_n[0] % 2 == 0:
                        nc.vector.tensor_copy(ds[:, 1], tph[:, kb])
                        nc.scalar.activation(ds[:, 0], tpl[:, kb], AF.Copy,
                                             scale=16.0)
                    else:
                        nc.scalar.copy(ds[:, 1], tph[:, kb])
                        nc.vector.tensor_scalar_mul(ds[:, 0], tpl[:, kb],
                                                    16.0)
                    rst_n[0] += 1
                # um quarter-copies ride the acts of the last four groups:
                # transfers use phase-1 DMA slack instead of fighting the
                # phase-2 support prefetch right at the boundary.
                if g >= NG - 4:
                    q = g - (NG - 4)
                    nc.gpsimd.dma_start(
                        um0_tiles[i][:, q * N // 4:(q + 1) * N // 4],
                        ruT_tiles[i][OUT:2 * OUT, q * N // 4:(q + 1) * N // 4])

            def v_drain(dst, src_ap):
                nc.vector.tensor_scalar_mul(dst, src_ap, 1.0 / 16.0)

            def s_drain(dst, src_ap):
                nc.scalar.activation(dst, src_ap, AF.Copy, scale=1.0 / 16.0)

            big_phase(
                xs_t, wru_t, 2 * OUT, bru_ap, AF.Sigmoid, sgn_ap,
                lambda i, k0, w: ruT_tiles[i][:, k0:k0 + w],
                agg_rr=lambda n: v_drain if n % 2 == 0 else s_drain,
                aggT_rr=[nc.scalar.copy, nc.vector.tensor_copy],
                extra_fn=phase1_extra,
                post_act=phase1_post_act,
                tp_bufs=4,
                pre_fn=xs_load, post_fn=xs_load_rest)

            # ---------------- phase 2 ----------------
            def phase2_post_act(i, g, pj_pool, kb=None):
                # out chunk = state + um*(c - state), in place over stT.
                # Whole chain on DVE: only same-queue deps -> never blocks
                # the drain stream at the queue head. kb: half-chunk mode
                # for the split final act.
                k0 = g * GW
                w = GW
                if kb is not None:
                    k0, w = g * GW + kb * P, P
                c_ap = ruT_tiles[i][0:OUT, k0:k0 + w]
                s_ap = stT_tiles[i][:, k0:k0 + w]
                d = cscr_pool.tile([OUT, GW], f16, tag="cscr")
                nc.vector.tensor_sub(d[:, 0:w], c_ap, s_ap)
                t = cscr_pool.tile([OUT, GW], f16, tag="cscr")
                nc.vector.tensor_mul(t[:, 0:w], um0_tiles[i][:, k0:k0 + w],
                                     d[:, 0:w])
                nc.vector.tensor_add(s_ap, s_ap, t[:, 0:w])
                if g == NG - 2:
                    # store all but the last chunk early; shrinks the tail
                    nc.sync.dma_start(outT_d[i][:, 0:(NG - 1) * GW],
                                      stT_tiles[i][:, 0:(NG - 1) * GW])
                elif g == NG - 1:
                    nc.sync.dma_start(outT_d[i][:, k0:k0 + w],
                                      stT_tiles[i][:, k0:k0 + w])

            big_phase(
                xc_t, wc_t, OUT, bc_ap, AF.Tanh, 1.0,
                lambda i, k0, w: ruT_tiles[i][0:OUT, k0:k0 + w],
                agg_rr=lambda n: s_drain if n % 2 == 0 else v_drain,
                aggT_rr=[nc.scalar.copy, nc.vector.tensor_copy],
                post_act=phase2_post_act,
                tp_bufs=4, defer_first=True)

    nc.compile()
    return nc


def _get_module():
    if "nc" not in _CACHE:
        _CACHE["nc"] = _build_module()
    return _CACHE["nc"]


def pack_inputs(input, state, supports, Wr, br, Wu, bu, Wc, bc):
    """Host-side packing shared by kernel() and local sim harnesses."""
    input = np.asarray(input, np.float32)
    state = np.asarray(state, np.float32)
    supports = np.asarray(supports, np.float32)
    Wr = np.asarray(Wr, np.float32)
    br = np.asarray(br, np.float32)
    Wu = np.asarray(Wu, np.float32)
    bu = np.asarray(bu, np.float32)
    Wc = np.asarray(Wc, np.float32)
    bc = np.asarray(bc, np.float32)

    f8 = ml_dtypes.float8_e4m3

    # ST[j][m, k] = S[j][k, m] * SCALE; hi/lo split for the 3-product
    # compensated aggregation; swizzled [j, g, p, mt, slot(Sh16|Sh|Sl), k']
    stf = supports.transpose(0, 2, 1) * SCALE
    sh = stf.astype(f8)
    sl = (16.0 * (stf - sh.astype(np.float32))).astype(f8)
    sh16 = (sh.astype(np.float32) * 16.0).astype(f8)

    def _st_sw(a):
        # [J, m, k] -> [J, g, p, mt, k'] with m = mt*128 + p
        return a.reshape(J, 2 * MQ, P, NG, GW).transpose(0, 3, 2, 1, 4)

    st_host = np.stack([_st_sw(sh16), _st_sw(sh), _st_sw(sl)], axis=4)
    st_host = np.ascontiguousarray(
        st_host.reshape(J, NG, P, 2, MQ, 3, GW).transpose(
            0, 1, 3, 2, 4, 5, 6))

    # wcat: [66, J*128 | J*64] = [Wr|Wu] per j then Wc per j, all / SCALE
    wru = (np.concatenate([Wr, Wu], axis=2) / SCALE).transpose(1, 0, 2)
    wcc = (Wc / SCALE).transpose(1, 0, 2)
    wcat = np.ascontiguousarray(np.concatenate(
        [wru.reshape(F, J * 2 * OUT), wcc.reshape(F, J * OUT)],
        axis=1)).astype(np.float16)

    # bvec: col0 = [br | -bu], col1 = [+1|-1], col2 = [bc | 0]
    bvec = np.zeros((P, 3), np.float32)
    bvec[:, 0] = np.concatenate([br, -bu])
    bvec[:, 1] = np.concatenate([np.ones(OUT), -np.ones(OUT)])
    bvec[0:OUT, 2] = bc

    xs_full = np.concatenate([input, state], axis=2)  # [B, N, F]

    in_maps = []
    for c in range(NCORES):
        csl = slice(c * BC, (c + 1) * BC)
        # [N, Bc, F] -> [p, h, mt, slot(xl|xh), 264] with m = mt*128 + p
        xsc = xs_full[csl].transpose(1, 0, 2).reshape(2 * MQ, P, 2, HALF)
        xh = xsc.astype(f8)
        xl = (16.0 * (xsc - xh.astype(np.float32))).astype(f8)
        xs_c = np.ascontiguousarray(
            np.stack([xl, xh], axis=3).transpose(1, 2, 0, 3, 4))
        stT_c = np.ascontiguousarray(
            state[csl].transpose(0, 2, 1)).astype(np.float16)
        in_maps.append({
            "st": st_host,
            "xs": xs_c,
            "stT": stT_c,
            "wcat": wcat,
            "bvec": bvec,
        })
    return in_maps


def kernel(input, state, supports, Wr, br, Wu, bu, Wc, bc):
    from concourse.bass_utils import run_bass_kernel_spmd

    nc = _get_module()
    in_maps = pack_inputs(input, state, supports, Wr, br, Wu, bu, Wc, bc)

    import time
    t0 = time.monotonic()
    res = run_bass_kernel_spmd(nc, in_maps, core_ids=list(range(NCORES)))
    _CACHE["last_wall_s"] = time.monotonic() - t0

    out = np.empty((B, N, OUT), np.float32)
    for c in range(NCORES):
        outT = res.results[c]["outT"]           # [BC, OUT, N] f16
        out[c * BC:(c + 1) * BC] = outT.transpose(0, 2, 1).astype(np.float32)
    return out

